# revision 37
# baseline (speedup 1.0000x reference)
"""NT-Xent contrastive loss on 8 Trainium2 NeuronCores (bf16 + XBAR).

Math (reference): z = [z_i; z_j] (N=8192, D=128), zn = z/||z||,
sim = zn@zn.T / 0.1.  Row loss_i = logsumexp_{j!=i} sim[i,j] - sim[i, pos(i)],
loss = mean_i loss_i.

Sharding: rolled-column trick.  Core c receives z rolled by -1024*c rows.
Its 1024 local rows are rolled rows 0..1023; in rolled coordinates the
self column of local row i is i and the positive column is i + 4096 on
EVERY core, so a single static SPMD program works with no collectives.
The self logit is suppressed by adding -5 to the diagonal cosine
(logit -40 -> exp ~4e-18, negligible).  Host sums the 8 partial means.

V5 design notes (hard-won via perfetto):
  - ACT exp stream is the floor (1 elem/cycle/lane @1.2GHz, dtype
    independent).  DVE_W of each 2048-col chunk is exp'd on DVE via a
    16-bit Schraudolph: i16 = int(A*cos + B) bitcast to bf16 IS
    exp(10*cos) to ~2%; a second DVE pass reduces RGRP chunks' bf16
    views at once (TENSOR_REDUCE is 1x even all-2B; tensor_scalar
    accum_out dies on HW).
  - ALL DMAs (both hwdge queues + gpsimd swdge) retire in the tile
    scheduler's simulated global order via counting semaphores, with
    ~2us latency per cross-engine hop.  So: one merged const DMA, two
    transfers per late batch, batches 1-3 dep-pinned AFTER the prologue
    xbars (issue order alone put them first and stalled the xbars ~10us
    behind 12 input transfers).
  - DMA reads are whole-tensor granular: zn is a per-half tensor or the
    half-0 xbar waits for all four zn scale writes.
  - Batch b stage-A is dep-pinned after batch b-1's last xbar:
    priorities cannot stop the scheduler from hoisting it into the
    prologue (its DMA model is optimistic), poisoning counter waits.
Known-dead ends (trace-verified): collectives (~87us), 3-pass f32
Schraudolph, xbar on the scalar queue, tensor_scalar accum_out on HW,
gating late DMAs behind gpsimd dummy-copies (scheduler ignores it).
"""

import os
import sys

import numpy as np

_TRN_REPO = "/opt/trn_rl_repo"
if _TRN_REPO not in sys.path:
    sys.path.insert(0, _TRN_REPO)

from concourse import bacc, bass, mybir, tile
from concourse.bass_utils import run_bass_kernel_spmd
from concourse.tile_rust import add_dep_helper

B = 4096
D = 128
N = 2 * B
N_CORES = 8
RPC = N // N_CORES  # 1024 rows per core
INV_T = 10.0
DIAG_SHIFT = -5.0

NBATCH = 4  # stage-A batches of 2048 rows
TPB = 16    # 128-row tiles per batch
RB = 8      # row blocks per core (128 rows each)
QB = 4      # 2048-wide column chunks
KB = 4      # 512-wide matmuls per chunk

DVE_W = 384             # columns of each chunk exp'd on DVE (Schraudolph)
ACT_W = 2048 - DVE_W    # columns exp'd on ACT
RGRP = 2                # chunks per batched Schraudolph reduce
# Schraudolph constants: bf16 bit pattern of exp(10*x) ~= int(A*x + B)
SCH_A = INV_T * 128.0 / float(np.log(2.0))
SCH_B = 127.0 * 128.0 - 7.4

_cache: dict = {}


def build():
    f32 = mybir.dt.float32
    bf16 = mybir.dt.bfloat16
    i16 = mybir.dt.int16
    AX = mybir.AxisListType
    AF = mybir.ActivationFunctionType
    ALU = mybir.AluOpType

    nc = bacc.Bacc(
        "TRN2", target_bir_lowering=False, debug=False, num_devices=N_CORES
    )

    # Pin ln/exp/copy/etc to one ACT table: avoids 1.3us ACT_TABLE_LOAD at
    # every ln<->exp transition.
    tabs = bacc.get_activation_tables(nc.m.arch)
    pinned = set(tabs["natural_log_exp_and_others"])
    for k in tabs:
        if k != "natural_log_exp_and_others":
            tabs[k] = tabs[k] - pinned

    # z arrives as bf16 (host casts during the roll/shard prep): halves the
    # HBM read traffic all 8 cores contend for.
    z_dram = nc.dram_tensor("z_roll", [N, D], bf16, kind="ExternalInput")
    loss_dram = nc.dram_tensor("loss_part", [1, 1], f32, kind="ExternalOutput")

    eye_np = np.eye(128, dtype=np.float32)
    ones_dram = nc.inline_tensor(np.ones((128, 1), np.float32), name="ones")
    import ml_dtypes
    bfc_np = np.concatenate([eye_np, DIAG_SHIFT * eye_np], axis=1)
    bfc_dram = nc.inline_tensor(
        bfc_np.astype(ml_dtypes.bfloat16), name="bfconsts"
    )

    with tile.TileContext(nc) as tc:
        with (
            tc.tile_pool(name="const", bufs=1) as cpool,
            tc.tile_pool(name="zin", bufs=NBATCH) as zpool,
            tc.tile_pool(name="zn", bufs=4) as npool,
            tc.tile_pool(name="persist", bufs=1) as ppool,
            tc.tile_pool(name="scr", bufs=2) as spool,
            tc.tile_pool(name="psum", bufs=2, space=bass.MemorySpace.PSUM) as qpool,
        ):
            ones_sb = cpool.tile([128, 1], f32)
            bfc_sb = cpool.tile([128, 256], bf16)
            ident_bf = bfc_sb[:, 0:128]
            negI_bf = bfc_sb[:, 128:256]

            ssq = ppool.tile([128, NBATCH * TPB], f32)
            lnssq = ppool.tile([128, NBATCH * TPB], f32)
            inv = ppool.tile([128, NBATCH * TPB], f32)
            # znT as per-half tensors: DMA writes are whole-tensor granular,
            # so per-half tensors let matmuls start after half 0's xbar.
            znT = [
                [
                    ppool.tile([128, 1024], bf16, name=f"znT{b}h{h}")
                    for h in range(2)
                ]
                for b in range(NBATCH)
            ]
            sexp = ppool.tile([128, RB, QB, 2], f32)
            comb2 = ppool.tile([128, 2], f32)

            # --- input DMAs ---
            # batch 0 as four separate 512-row tensors: the per-piece stage-A
            # chain starts as each piece lands instead of waiting for the
            # whole batch.  Constants ride the gpsimd queue first (tiny).
            zin0 = [
                zpool.tile([128, 4, 128], bf16, name=f"zin0s{s}")
                for s in range(4)
            ]
            b0_engs = [nc.sync, nc.scalar, nc.gpsimd, nc.sync]
            for s in range(4):
                r0 = 512 * s
                src = z_dram[r0 : r0 + 512, :].rearrange(
                    "(t p) d -> p t d", p=128
                )
                b0_engs[s].dma_start(zin0[s][:], src)
            nc.gpsimd.dma_start(bfc_sb[:], bfc_dram[:])
            nc.gpsimd.dma_start(ones_sb[:], ones_dram[:])

            def bc(iv):
                # broadcast [128, t] -> [128, t, 128] via stride-0 last dim
                return bass.AP(iv.tensor, iv.offset, iv.ap + [[0, 128]])

            # --- prologue: batch 0 stage-A issued in dependency order per
            # half so the scheduler keeps the critical chain tight ---
            # zn as per-half tensors: the xbar (a DMA) read-tracks whole
            # tensors, so a single zn would wait for all four scale writes.
            zn_tiles = {}
            for h in range(2):
                zn_tiles[(0, h)] = npool.tile(
                    [128, 8, 128], bf16, name=f"zn0h{h}", tag="zn"
                )
            trans_insts = {}
            with tc.high_priority():
                for h in range(2):
                    for s in (2 * h, 2 * h + 1):
                        scr_s = spool.tile([128, 4 * 128], bf16, tag="sqp")
                        zv = zin0[s][:].rearrange("p t d -> p (t d)")
                        nc.vector.tensor_mul(scr_s[:], zv, zv)
                        nc.vector.reduce_sum(
                            ssq[:, 4 * s : 4 * s + 4],
                            scr_s[:].rearrange("p (t d) -> p t d", d=128),
                            axis=AX.X,
                        )
                    # norms for this half: 1/||z|| = exp(-0.5*ln(ssq))
                    j0, j1 = 8 * h, 8 * h + 8
                    nc.scalar.activation(
                        lnssq[:, j0:j1], ssq[:, j0:j1], AF.Ln
                    )
                    nc.scalar.activation(
                        inv[:, j0:j1], lnssq[:, j0:j1], AF.Exp, scale=-0.5
                    )
                    for s in (2 * h, 2 * h + 1):
                        nc.vector.tensor_mul(
                            zn_tiles[(0, h)][:, 4 * (s - 2 * h) : 4 * (s - 2 * h) + 4, :],
                            zin0[s][:],
                            bc(inv[:, 4 * s : 4 * s + 4]),
                        )
                    # h1 rides the idle scalar queue so both prologue
                    # xbars run in parallel (the V2-era hazard was the
                    # stage-A inversion, fixed by dep-pins since).
                    xeng = nc.sync if h == 0 else nc.scalar
                    trans_insts[(0, h)] = xeng.dma_start_transpose(
                        znT[0][h][:].rearrange("p (t c) -> p t c", c=128),
                        zn_tiles[(0, h)][:].rearrange("p t d -> p (t d)"),
                    )

            # batches 1-3: two 1024-row transfers each on gpsimd, dep-pinned
            # after the prologue xbars so the scheduler's global DMA order
            # cannot put them first (which stalls the xbars ~10us).
            zin_tiles = {}
            for b in range(1, NBATCH):
                zin_tiles[b] = zpool.tile(
                    [128, TPB, 128], bf16, name=f"zin{b}"
                )
            for b in range(1, NBATCH):
                for hh in range(2):
                    r0 = 2048 * b + 1024 * hh
                    src = z_dram[r0 : r0 + 1024, :].rearrange(
                        "(t p) d -> p t d", p=128
                    )
                    ins = nc.sync.dma_start(
                        zin_tiles[b][:, 8 * hh : 8 * hh + 8, :], src
                    )
                    add_dep_helper(
                        ins.ins,
                        trans_insts[(0, 1)].ins,
                        sync=True,
                        reason="late input DMAs after prologue xbars",
                    )

            # --- in-loop stage-A helpers for batches 1-3 ---
            def ssq_mul(b):
                # gpsimd is idle in-loop (inputs ride the sync queue); its
                # 0.42-efficiency Multiply (~3.5us) is free wall-clock and
                # takes the biggest stage-A burst off the DVE.
                scr = spool.tile([128, TPB * 128], bf16, tag="sq")
                zv = zin_tiles[b][:].rearrange("p t d -> p (t d)")
                ins = nc.vector.tensor_mul(scr[:], zv, zv)
                # hard-pin after the previous batch's last xbar: priorities
                # cannot stop the scheduler from hoisting this into the
                # prologue, which poisons every later counter-based wait.
                add_dep_helper(
                    ins.ins,
                    trans_insts[(b - 1, 1)].ins,
                    sync=True,
                    reason="stage-A after prev batch xbar",
                )
                return scr

            def ssq_red(b, scr, h):
                j0 = TPB * b + 8 * h
                nc.vector.reduce_sum(
                    ssq[:, j0 : j0 + 8],
                    scr[:, 1024 * h : 1024 * (h + 1)].rearrange(
                        "p (t d) -> p t d", d=128
                    ),
                    axis=AX.X,
                )

            def norms(b):
                j0 = TPB * b
                nc.scalar.activation(
                    lnssq[:, j0 : j0 + TPB], ssq[:, j0 : j0 + TPB], AF.Ln
                )
                nc.scalar.activation(
                    inv[:, j0 : j0 + TPB], lnssq[:, j0 : j0 + TPB],
                    AF.Exp, scale=-0.5,
                )

            def tsm(b, h):
                zn_tiles[(b, h)] = npool.tile(
                    [128, 8, 128], bf16, name=f"zn{b}h{h}", tag="zn"
                )
                t0 = 8 * h
                nc.vector.tensor_mul(
                    zn_tiles[(b, h)][:],
                    zin_tiles[b][:, t0 : t0 + 8, :],
                    bc(inv[:, TPB * b + t0 : TPB * b + t0 + 8]),
                )

            def build_trans(b, h):
                trans_insts[(b, h)] = nc.sync.dma_start_transpose(
                    znT[b][h][:].rearrange("p (t c) -> p t c", c=128),
                    zn_tiles[(b, h)][:].rearrange("p t d -> p (t d)"),
                )

            # --- main loop: q-outer, r-inner; build batch q+1 under chunk q ---
            # Pre-allocate the PSUM ring so chunk i can aim PE-warmup dummy
            # matmuls at chunk i+1's buffer (start=True overwrites them).
            ps_ring = [
                qpool.tile([128, 2048], f32, tag="mm", name=f"ps{i}")
                for i in range(QB * RB)
            ]
            # Prologue PE-HAM warmup: ~7us of contiguous junk matmuls (>3.4us
            # busy window) guarantees the 2.4GHz latch before the loop; the
            # in-loop dummies then keep every HAM window non-idle.  Source is
            # a memset tile; target is chunk 0's buffer (start=True real
            # matmuls overwrite it).
            junk = cpool.tile([128, 512], bf16)
            nc.vector.memset(junk[:], 0)
            for _ in range(16):
                nc.tensor.matmul(
                    ps_ring[0][:, 0:512],
                    junk[:, 0:128],
                    junk[:],
                    start=True,
                    stop=True,
                    skip_group_check=True,
                )
            e16 = None
            for q in range(QB):
                b = q + 1
                for r in range(RB):
                    i = q * RB + r
                    lhsT = znT[0][0][:, 128 * r : 128 * (r + 1)]
                    ps = ps_ring[i]
                    for k in range(KB):
                        rhs = znT[q][k // 2][:, 512 * (k % 2) : 512 * (k % 2 + 1)]
                        nc.tensor.matmul(
                            ps[:, 512 * k : 512 * (k + 1)],
                            lhsT,
                            rhs,
                            start=True,
                            stop=True,
                        )
                    # PE-HAM warmup: keep the PE busy through the exp phase
                    # so the activity monitor holds the 2.4GHz clock; the
                    # dummies land in the next chunk's buffer, which its
                    # real start=True matmuls overwrite.
                    if i + 1 < QB * RB:
                        for _ in range(3):
                            nc.tensor.matmul(
                                ps_ring[i + 1][:, 0:512],
                                lhsT,
                                znT[q][0][:, 0:512],
                                start=True,
                                stop=True,
                                skip_group_check=True,
                            )
                    if q == 0:
                        # diag shift on the PE: accumulate I^T @ (-5 I)
                        # onto the diag sub-block (start=False).  No DVE
                        # op, no cross-engine hop before the exp.
                        nc.tensor.matmul(
                            ps[:, 128 * r : 128 * (r + 1)],
                            ident_bf,
                            negI_bf,
                            start=False,
                            stop=True,
                            skip_group_check=True,
                        )
                    last = q == QB - 1 and r == RB - 1
                    # q3 has no stage-A on the DVE: shift more exp columns
                    # to it there (both engines land ~1.6us/chunk).
                    wq = 640 if q == QB - 1 else DVE_W
                    aw = 2048 if last else 2048 - wq
                    # ACT: exp with fused row-sum accumulate.  The final
                    # chunk runs fully on ACT so no Schraudolph pass or
                    # reduce sits in the tail's serial chain.
                    nc.scalar.activation(
                        ps[:, 0:aw],
                        ps[:, 0:aw],
                        AF.Exp,
                        scale=INV_T,
                        accum_out=sexp[:, r, q, 0:1],
                    )
                    g = r % RGRP
                    if not last:
                        if g == 0:
                            e16 = spool.tile(
                                [128, RGRP, 640], i16, tag="sch"
                            )
                        nc.vector.tensor_scalar(
                            e16[:, g, 0:wq],
                            ps[:, aw:2048],
                            SCH_A,
                            SCH_B,
                            op0=ALU.mult,
                            op1=ALU.add,
                        )
                    if g == RGRP - 1 and not last:
                        nc.vector.reduce_sum(
                            sexp[:, r - RGRP + 1 : r + 1, q, 1:2],
                            e16[:, :, 0:wq].bitcast(bf16),
                            axis=AX.X,
                        )
                    elif last:
                        nc.vector.reduce_sum(
                            sexp[:, r - 1 : r, q, 1:2],
                            e16[:, 0:1, 0:wq].bitcast(bf16),
                            axis=AX.X,
                        )
                        nc.vector.memset(sexp[:, r : r + 1, q, 1:2], 0.0)
                    if q == 3 and r == 0:
                        with tc.high_priority(offset=-100000):
                            # pos_i = zn_i . zn_{i+4096}: one elementwise
                            # mul of znT[0]h0 x znT[2]h0 plus a full
                            # reduce -- only sum(pos) enters the loss.
                            pp = spool.tile([128, 1024], bf16, tag="pp")
                            nc.vector.tensor_mul(
                                pp[:], znT[0][0][:], znT[2][0][:]
                            )
                            ppr = ppool.tile([128, 1], f32)
                            nc.vector.reduce_sum(ppr[:], pp[:], axis=AX.X)
                            nc.vector.tensor_scalar(
                                comb2[:, 1:2], ppr[:], -INV_T, None,
                                op0=ALU.mult,
                            )
                    if b < NBATCH:
                        with tc.high_priority(offset=-100000):
                            if r == 0:
                                zn_tiles[f"scr{b}"] = ssq_mul(b)
                            elif r == 1:
                                ssq_red(b, zn_tiles[f"scr{b}"], 0)
                            elif r == 2:
                                ssq_red(b, zn_tiles[f"scr{b}"], 1)
                            elif r == 3:
                                norms(b)
                            elif r == 4:
                                tsm(b, 0)
                                tsm(b, 1)
                            elif r == 5:
                                build_trans(b, 0)
                            elif r == 6:
                                build_trans(b, 1)

            # --- epilogue: minimal serial chain ---
            # s8 = per-rowblock sums over (q, act|dve); lnsum = sum_r ln(s8)
            # via accum_out into comb2 col 0 (col 1 = -10*possum, built
            # during q3); one ones-matmul partition-reduces both columns;
            # two tiny DVE ops finish on one partition.
            s8 = ppool.tile([128, RB], f32)
            nc.vector.reduce_sum(
                s8[:], sexp[:].rearrange("p r q t -> p r (q t)"), axis=AX.X
            )
            lse = ppool.tile([128, RB], f32)
            nc.scalar.activation(
                lse[:], s8[:], AF.Ln, accum_out=comb2[:, 0:1]
            )
            nc.tensor.matmul(
                ps_ring[30][0:1, 0:2], ones_sb[:], comb2[:],
                start=True, stop=True, skip_group_check=True,
            )
            red1 = ppool.tile([1, 1], f32)
            nc.vector.reduce_sum(
                red1[:], ps_ring[30][0:1, 0:2].rearrange("p (a b) -> p a b", a=1),
                axis=AX.X,
            )
            res = ppool.tile([1, 1], f32)
            nc.vector.tensor_scalar(
                res[:], red1[:], 1.0 / N, None, op0=ALU.mult
            )
            nc.gpsimd.dma_start(loss_dram[:], res[:])

    nc.compile()
    return nc


def get_nc():
    if "nc" not in _cache:
        _cache["nc"] = build()
    return _cache["nc"]


def make_in_maps(z_i: np.ndarray, z_j: np.ndarray):
    import ml_dtypes

    z = np.concatenate(
        [np.asarray(z_i, np.float32), np.asarray(z_j, np.float32)], axis=0
    ).astype(ml_dtypes.bfloat16)
    return [
        {"z_roll": np.ascontiguousarray(np.roll(z, -RPC * c, axis=0))}
        for c in range(N_CORES)
    ]


def kernel(**inputs) -> np.ndarray:
    in_maps = make_in_maps(inputs["z_i"], inputs["z_j"])
    nc = get_nc()
    res = run_bass_kernel_spmd(nc, in_maps, list(range(N_CORES)))
    kernel.last_results = res
    total = np.float32(0.0)
    for r in res.results:
        total = np.float32(total + np.float32(np.asarray(r["loss_part"]).reshape(())))
    return np.float32(total)
